# revision 21
# baseline (speedup 1.0000x reference)
"""Trainium2 Bass kernel for nn_FactorMask (9-tap masked-stencil op).

Contract: kernel(**inputs) takes FULL inputs (inp [8,224,224,32] f32,
kernel [9,1,1,1,32], mask [9,1,1,1,32]) and returns the FULL output
[8,224,224,32] f32. Internally: batch-parallel across 8 NeuronCores
(1 image per core), host-side repack to a channels-on-partition layout,
Bass/Tile kernel per core, host-side gather.

Layout per core: xh [128, 58, 226] fp16 where partition p = 32*q + c
(q = H-quarter 0..3, c = channel), rows = 58 padded rows of that quarter
(56 output rows + 1 halo row each side), cols = 226 zero-padded W.
All 9 stencil taps are then pure free-dim AP offsets.

Math (per pixel/channel): A_e = |m_e x_e - k_e|, mu = (sum m_e x_e)/9 - kbar,
norm1 = sum A_e, var = sum |A_e - mu|. Using |A-mu| = A + mu - 2*min(A,mu):
    var = norm1 + 9 mu - 2 S,   S = sum_e min(A_e, mu)
    out = (1 - var/9)(1 - norm1/9) = W * (W - mu + (2/9) S),  W = (9-norm1)/9
so the var path needs only one 2x tensor_tensor(min) per tap instead of
subtract+abs.

Work split (v3):
- Act: A_e for taps 0..5 (fused abs-affine), mean affine, W fold
- DVE: A_e for taps 6..8 (two 4x tensor_scalar ops), min-fields (2x),
  optional pair pre-sums, final combine
- PE: mean conv (9 diag matmuls) + norm1 + S sums per segment
"""

import os
import sys

for _p in ("/opt/trn_rl_repo", "/opt/pypackages"):
    if _p not in sys.path:
        sys.path.insert(0, _p)

import numpy as np

import concourse.bacc as bacc
import concourse.mybir as mybir
import concourse.tile as tile
from concourse.bass_utils import run_bass_kernel_spmd

# ---- problem constants (hardcoded per the task spec) ----
B, H, W, C = 8, 224, 224, 32
E = 9  # taps
NCORES = 8
Q = 4  # H-quarters per image -> 4*32 = 128 partitions
RQ = H // Q  # 56 output rows per quarter
RA = RQ + 2  # 58 rows incl halo
WP = W + 2  # 226 padded cols
P = 128

# tap order must match reference: element 0 = center, then (y,x) raster
# order skipping center, with shift (dy,dx) = (y-1, x-1)
TAPS = [(0, 0)] + [
    (dy, dx) for dy in (-1, 0, 1) for dx in (-1, 0, 1) if not (dy == 0 and dx == 0)
]

# ---- tunables ----
SEG = 2  # rows per matmul segment (2*224 = 448 <= 512)
NDVE = int(os.environ.get("FM_NDVE", "2"))  # taps whose A is made on DVE
NPAIR_A = int(os.environ.get("FM_NPAIR_A", "1"))  # DVE pre-sum pairs (norm1)
NPAIR_S = int(os.environ.get("FM_NPAIR_S", "2"))  # DVE pre-sum pairs (S)
F32 = mybir.dt.float32
HDT = mybir.dt.float16
U16 = mybir.dt.uint16

_CACHE = {}


def _build_program():
    nc = bacc.Bacc(
        "TRN2", target_bir_lowering=False, debug=False, num_devices=NCORES
    )
    xh_d = nc.dram_tensor("xh", [P, RA, WP], HDT, kind="ExternalInput").ap()
    # pv columns: [0:9]=m_e scale, [9:18]=-k_e bias, [18]=-kbar
    pv_d = nc.dram_tensor("pv", [P, 2 * E + 1], F32, kind="ExternalInput").ap()
    # wm[:, e, :] = diag(m_e) for e<9 ; wm[:, 9, :] = identity  (fp16)
    wm_d = nc.dram_tensor("wm", [P, E + 1, P], HDT, kind="ExternalInput").ap()
    y_d = nc.dram_tensor("y", [P, RQ, W], HDT, kind="ExternalOutput").ap()

    with tile.TileContext(nc) as tc:
        _emit(tc, nc, xh_d, pv_d, wm_d, y_d)
    nc.compile()
    return nc


def _band_sizes():
    sched = os.environ.get("FM_BANDS", "2,4,8,8,8,8,8,8,2")
    sizes = [int(s) for s in sched.split(",")]
    assert sum(sizes) == RQ and all(s % SEG == 0 for s in sizes)
    return sizes


def _emit(tc, nc, xh_d, pv_d, wm_d, y_d):
    Abs = mybir.ActivationFunctionType.Abs
    Ident = mybir.ActivationFunctionType.Identity
    Copy = mybir.ActivationFunctionType.Copy
    sub = mybir.AluOpType.subtract
    mult = mybir.AluOpType.mult
    add = mybir.AluOpType.add
    amin = mybir.AluOpType.min
    band_ = mybir.AluOpType.bitwise_and

    a_pairs = [(1 + 2 * i, 2 + 2 * i) for i in range(NPAIR_A)]
    s_pairs = [(1 + 2 * i, 2 + 2 * i) for i in range(NPAIR_S)]
    a_paired = {t for p in a_pairs for t in p}
    s_paired = {t for p in s_pairs for t in p}

    with (
        tc.tile_pool(name="const", bufs=1) as cpool,
        tc.tile_pool(name="xin", bufs=3) as xpool,
        tc.tile_pool(name="work", bufs=2) as wpool,
        tc.tile_pool(name="acc", bufs=2, space="PSUM") as ppool,
    ):
        # Startup DMAs issue from separate engine queues so they don't
        # serialize on Sync: wm gates the first LDWEIGHTS, xh band 0/1 gate
        # the first matmuls/activations.
        wm = cpool.tile([P, E + 1, P], HDT)
        nc.sync.dma_start(wm[:], wm_d[:])
        pv = cpool.tile([P, 2 * E + 1], F32)
        nc.sync.dma_start(pv[:], pv_d[:])

        r0 = 0
        for band, rb in enumerate(_band_sizes()):
            nseg = rb // SEG
            xbh = xpool.tile([P, rb + 2, WP], HDT, tag="xbh")
            nc.sync.dma_start(xbh[:], xh_d[:, r0 : r0 + rb + 2, :])

            # mean sums per segment (PE accumulation groups) — emitted first
            # so the min-phase unblocks as early as possible
            mean = wpool.tile([P, rb, W], HDT, tag="mean")
            for s in range(nseg):
                i0 = s * SEG
                sp = ppool.tile([P, SEG, W], F32, tag="sp")
                for e, (dy, dx) in enumerate(TAPS):
                    rhs = xbh[
                        :, i0 + 1 + dy : i0 + 1 + dy + SEG, 1 + dx : 1 + dx + W
                    ]
                    nc.tensor.matmul(
                        sp[:],
                        wm[:, e, :],
                        rhs,
                        start=(e == 0),
                        stop=(e == E - 1),
                    )
                # mean = sp/9 - kbar   (Act, PSUM in)
                nc.scalar.activation(
                    mean[:, i0 : i0 + SEG, :],
                    sp[:],
                    Ident,
                    bias=pv[:, 2 * E : 2 * E + 1],
                    scale=1.0 / E,
                )

            # A_e = |m_e * x - k_e| on the tap's shifted window, so all
            # downstream reads are dense/aligned. The last NDVE taps are made
            # on DVE (two 4x tensor_scalar ops via a signed-g intermediate).
            A = []
            for e, (dy, dx) in enumerate(TAPS):
                a = wpool.tile([P, rb, W], HDT, tag=f"A{e}")
                win = xbh[:, 1 + dy : 1 + dy + rb, 1 + dx : 1 + dx + W]
                if e < E - NDVE:
                    nc.scalar.activation(
                        a[:],
                        win,
                        Abs,
                        bias=pv[:, E + e : E + e + 1],
                        scale=pv[:, e : e + 1],
                    )
                else:
                    # signed affine into the A tile, then in-place abs via
                    # sign-bit clear (both 4x tensor_scalar)
                    nc.vector.tensor_scalar(
                        a[:],
                        win,
                        pv[:, e : e + 1],
                        pv[:, E + e : E + e + 1],
                        mult,
                        add,
                    )
                    nc.vector.tensor_scalar(
                        a[:].bitcast(U16), a[:].bitcast(U16), 0x7FFF, None, band_
                    )
                A.append(a)

            # optional DVE pre-sums of A pairs (frees PE passes)
            APs = {}
            for i, (t0, t1) in enumerate(a_pairs):
                pa = wpool.tile([P, rb, W], HDT, tag=f"PA{i}")
                nc.vector.tensor_tensor(pa[:], A[t0][:], A[t1][:], add)
                APs[(t0, t1)] = pa
            norm_fields = [A[0]] + [APs[p] for p in a_pairs] + [
                A[e] for e in range(1, E) if e not in a_paired
            ]

            # norm1 = sum_e A_e on PE; W = (9 - norm1)/9 on Act.
            # Chains run per segment into bank-aligned halves of a 2-bank
            # PSUM tile; W folds once per segment-pair (fewer, bigger ops).
            SW = SEG * W
            w9 = wpool.tile([P, rb, W], HDT, tag="w9")
            for p0 in range(0, nseg, 2):
                nr = min(2, nseg - p0)
                sa2 = ppool.tile([P, 2, 512], F32, tag="sa", bufs=1)
                for j in range(nr):
                    i0 = (p0 + j) * SEG
                    for jf, f in enumerate(norm_fields):
                        nc.tensor.matmul(
                            sa2[:, j : j + 1, 0:SW],
                            wm[:, E, :],
                            f[:, i0 : i0 + SEG, :],
                            start=(jf == 0),
                            stop=(jf == len(norm_fields) - 1),
                        )
                nc.scalar.activation(
                    w9[:, p0 * SEG : (p0 + nr) * SEG, :],
                    sa2[:, 0:nr, 0:SW],
                    Copy,
                    bias=1.0,
                    scale=-1.0 / E,
                )

            # min-fields sm_e = min(A_e, mean)  (one 2x TT per tap)
            sm = []
            for e in range(E):
                t = wpool.tile(
                    [P, rb, W],
                    HDT,
                    tag=f"sm{e}",
                    bufs=int(os.environ.get("FM_SMBUFS", "2")),
                )
                nc.vector.tensor_tensor(t[:], A[e][:], mean[:], amin)
                sm.append(t)

            # in-place pair pre-sums: sm[t0] += sm[t1] (mins already done)
            for t0, t1 in s_pairs:
                nc.vector.tensor_tensor(sm[t0][:], sm[t0][:], sm[t1][:], add)
            s_fields = [sm[0]] + [sm[t0] for t0, _ in s_pairs] + [
                sm[e] for e in range(1, E) if e not in s_paired
            ]

            # S sum on PE, then out = W * (W - mean + (2/9) S), finals per
            # segment-pair on DVE
            ob = wpool.tile([P, rb, W], HDT, tag="ob")
            for p0 in range(0, nseg, 2):
                nr = min(2, nseg - p0)
                sv2 = ppool.tile([P, 2, 512], F32, tag="sv", bufs=2)
                for j in range(nr):
                    i0 = (p0 + j) * SEG
                    for jf, f in enumerate(s_fields):
                        nc.tensor.matmul(
                            sv2[:, j : j + 1, 0:SW],
                            wm[:, E, :],
                            f[:, i0 : i0 + SEG, :],
                            start=(jf == 0),
                            stop=(jf == len(s_fields) - 1),
                        )
                i0 = p0 * SEG
                i1 = (p0 + nr) * SEG
                # s1 = (2/9) S - mean   (STT, PSUM in)
                s1 = wpool.tile([P, 2 * SEG, W], HDT, tag="s1")
                nc.vector.scalar_tensor_tensor(
                    s1[:, 0 : i1 - i0, :],
                    sv2[:, 0:nr, 0:SW],
                    2.0 / E,
                    mean[:, i0:i1, :],
                    mult,
                    sub,
                )
                # s2 = W + s1 ; out = W * s2   (two 2x TTs)
                s2 = wpool.tile([P, 2 * SEG, W], HDT, tag="s2")
                nc.vector.tensor_tensor(
                    s2[:, 0 : i1 - i0, :], w9[:, i0:i1, :], s1[:, 0 : i1 - i0, :], add
                )
                nc.vector.tensor_tensor(
                    ob[:, i0:i1, :], w9[:, i0:i1, :], s2[:, 0 : i1 - i0, :], mult
                )
            nc.sync.dma_start(y_d[:, r0 : r0 + rb, :], ob[:])
            r0 += rb


def _host_pack(inp, kern, mask):
    """Build per-core input maps."""
    inp = np.ascontiguousarray(inp, dtype=np.float32)
    kern = np.asarray(kern, dtype=np.float32).reshape(E, C)
    mask = np.asarray(mask, dtype=np.float32).reshape(E, C)

    m = np.abs(mask) / (np.abs(mask).max() + np.float32(1e-6))  # [E,C]
    kbar = kern.mean(axis=0)  # [C]

    cidx = np.arange(P) % C
    pv = np.empty((P, 2 * E + 1), np.float32)
    for e in range(E):
        pv[:, e] = m[e][cidx]
        pv[:, E + e] = -kern[e][cidx]
    pv[:, 2 * E] = -kbar[cidx]

    wm = np.zeros((P, E + 1, P), np.float16)
    rng = np.arange(P)
    for e in range(E):
        wm[rng, e, rng] = m[e][cidx]
    wm[rng, E, rng] = 1.0

    in_maps = []
    for b in range(NCORES):
        padded = np.pad(inp[b], ((1, 1), (1, 1), (0, 0)))  # [226,226,32]
        # quarters: q needs padded rows [56q, 56q+58)
        qs = np.stack(
            [padded[RQ * q : RQ * q + RA] for q in range(Q)], axis=0
        )  # [4,58,226,32]
        x_dev = np.ascontiguousarray(
            qs.transpose(0, 3, 1, 2).reshape(P, RA, WP)
        )
        in_maps.append(
            {
                "xh": x_dev.astype(np.float16),
                "pv": pv,
                "wm": wm,
            }
        )
    return in_maps


def _host_unpack(results):
    out = np.empty((B, H, W, C), np.float32)
    for b in range(NCORES):
        y = results[b]["y"].astype(np.float32).reshape(Q, C, RQ, W)
        out[b] = y.transpose(0, 2, 3, 1).reshape(H, W, C)
    return out


LAST_PROFILE = {}


def _install_ntff_shim():
    """antenv.axon_hooks is missing in this image; synthesize it so
    run_bass_kernel_spmd(trace=True) can capture NTFF profiles."""
    import contextlib
    import ctypes
    import types

    if "antenv.axon_hooks" in sys.modules:
        return
    so_path = "/opt/axon/libaxon_pjrt.so"
    try:
        lib = ctypes.CDLL(so_path)
    except OSError:
        return
    if not hasattr(lib, "axon_start_nrt_profile"):
        return
    lib.axon_start_nrt_profile.argtypes = [
        ctypes.POINTER(ctypes.c_int64),
        ctypes.c_size_t,
    ]
    lib.axon_start_nrt_profile.restype = ctypes.c_int64
    lib.axon_stop_nrt_profile.argtypes = [ctypes.c_char_p]
    lib.axon_stop_nrt_profile.restype = ctypes.c_int64

    @contextlib.contextmanager
    def _hook(output_dir, device_ids):
        import jax

        jax.devices()
        if device_ids:
            ids = (ctypes.c_int64 * len(device_ids))(*device_ids)
            rc = lib.axon_start_nrt_profile(ids, len(device_ids))
        else:
            rc = lib.axon_start_nrt_profile(None, 0)
        if rc != 0:
            raise RuntimeError(f"axon_start_nrt_profile rc={rc}")
        try:
            yield
        finally:
            n = lib.axon_stop_nrt_profile(str(output_dir).encode())
            if n < 0:
                raise RuntimeError(f"axon_stop_nrt_profile rc={n}")
            print(f"ntff profile: {n} file(s) written to {output_dir}")

    mod = types.ModuleType("antenv.axon_hooks")
    mod._hook = _hook
    mod.get_axon_ntff_profile_hook = lambda: mod._hook
    mod.set_axon_ntff_profile_hook = lambda h: setattr(mod, "_hook", h)
    sys.modules["antenv.axon_hooks"] = mod


def kernel(inp, kernel, mask):
    if "nc" not in _CACHE:
        _CACHE["nc"] = _build_program()
    nc = _CACHE["nc"]

    in_maps = _host_pack(inp, kernel, mask)
    trace = bool(int(os.environ.get("FM_TRACE", "0")))
    if trace:
        _install_ntff_shim()
    res = run_bass_kernel_spmd(
        nc, in_maps, core_ids=list(range(NCORES)), trace=trace
    )
    LAST_PROFILE["exec_time_ns"] = res.exec_time_ns
    LAST_PROFILE["mean_exec_time_ns"] = res.mean_exec_time_ns
    return _host_unpack(res.results)


# revision 22
# speedup vs baseline: 1.0244x; 1.0244x over previous
"""Trainium2 Bass kernel for nn_FactorMask (9-tap masked-stencil op).

Contract: kernel(**inputs) takes FULL inputs (inp [8,224,224,32] f32,
kernel [9,1,1,1,32], mask [9,1,1,1,32]) and returns the FULL output
[8,224,224,32] f32. Internally: batch-parallel across 8 NeuronCores
(1 image per core), host-side repack to a channels-on-partition layout,
Bass/Tile kernel per core, host-side gather.

Layout per core: xh [128, 58, 226] fp16 where partition p = 32*q + c
(q = H-quarter 0..3, c = channel), rows = 58 padded rows of that quarter
(56 output rows + 1 halo row each side), cols = 226 zero-padded W.
All 9 stencil taps are then pure free-dim AP offsets.

Math (per pixel/channel): A_e = |m_e x_e - k_e|, mu = (sum m_e x_e)/9 - kbar,
norm1 = sum A_e, var = sum |A_e - mu|. Using |A-mu| = A + mu - 2*min(A,mu):
    var = norm1 + 9 mu - 2 S,   S = sum_e min(A_e, mu)
    out = (1 - var/9)(1 - norm1/9) = W * (W - mu + (2/9) S),  W = (9-norm1)/9
so the var path needs only one 2x tensor_tensor(min) per tap instead of
subtract+abs.

Work split (v3):
- Act: A_e for taps 0..5 (fused abs-affine), mean affine, W fold
- DVE: A_e for taps 6..8 (two 4x tensor_scalar ops), min-fields (2x),
  optional pair pre-sums, final combine
- PE: mean conv (9 diag matmuls) + norm1 + S sums per segment
"""

import os
import sys

for _p in ("/opt/trn_rl_repo", "/opt/pypackages"):
    if _p not in sys.path:
        sys.path.insert(0, _p)

import numpy as np

import concourse.bacc as bacc
import concourse.mybir as mybir
import concourse.tile as tile
from concourse.bass_utils import run_bass_kernel_spmd

# ---- problem constants (hardcoded per the task spec) ----
B, H, W, C = 8, 224, 224, 32
E = 9  # taps
NCORES = 8
Q = 4  # H-quarters per image -> 4*32 = 128 partitions
RQ = H // Q  # 56 output rows per quarter
RA = RQ + 2  # 58 rows incl halo
WP = W + 2  # 226 padded cols
P = 128

# tap order must match reference: element 0 = center, then (y,x) raster
# order skipping center, with shift (dy,dx) = (y-1, x-1)
TAPS = [(0, 0)] + [
    (dy, dx) for dy in (-1, 0, 1) for dx in (-1, 0, 1) if not (dy == 0 and dx == 0)
]

# ---- tunables ----
SEG = 2  # rows per matmul segment (2*224 = 448 <= 512)
NDVE = int(os.environ.get("FM_NDVE", "2"))  # taps whose A is made on DVE
NPAIR_A = int(os.environ.get("FM_NPAIR_A", "1"))  # DVE pre-sum pairs (norm1)
NPAIR_S = int(os.environ.get("FM_NPAIR_S", "2"))  # DVE pre-sum pairs (S)
F32 = mybir.dt.float32
HDT = mybir.dt.float16
U16 = mybir.dt.uint16

_CACHE = {}


def _build_program():
    nc = bacc.Bacc(
        "TRN2", target_bir_lowering=False, debug=False, num_devices=NCORES
    )
    xh_d = nc.dram_tensor("xh", [P, RA, WP], HDT, kind="ExternalInput").ap()
    # pv columns: [0:9]=m_e scale, [9:18]=-k_e bias, [18]=-kbar
    pv_d = nc.dram_tensor("pv", [P, 2 * E + 1], F32, kind="ExternalInput").ap()
    # wm[:, e, :] = diag(m_e) for e<9 ; wm[:, 9, :] = identity  (fp16)
    wm_d = nc.dram_tensor("wm", [P, E + 1, P], HDT, kind="ExternalInput").ap()
    y_d = nc.dram_tensor("y", [P, RQ, W], HDT, kind="ExternalOutput").ap()

    with tile.TileContext(nc) as tc:
        _emit(tc, nc, xh_d, pv_d, wm_d, y_d)
    nc.compile()
    return nc


def _band_sizes():
    sched = os.environ.get("FM_BANDS", "2,4,8,8,8,8,8,8,2")
    sizes = [int(s) for s in sched.split(",")]
    assert sum(sizes) == RQ and all(s % SEG == 0 for s in sizes)
    return sizes


def _emit(tc, nc, xh_d, pv_d, wm_d, y_d):
    Abs = mybir.ActivationFunctionType.Abs
    Ident = mybir.ActivationFunctionType.Identity
    Copy = mybir.ActivationFunctionType.Copy
    sub = mybir.AluOpType.subtract
    mult = mybir.AluOpType.mult
    add = mybir.AluOpType.add
    amin = mybir.AluOpType.min
    band_ = mybir.AluOpType.bitwise_and

    a_pairs = [(1 + 2 * i, 2 + 2 * i) for i in range(NPAIR_A)]
    s_pairs = [(1 + 2 * i, 2 + 2 * i) for i in range(NPAIR_S)]
    a_paired = {t for p in a_pairs for t in p}
    s_paired = {t for p in s_pairs for t in p}

    with (
        tc.tile_pool(name="const", bufs=1) as cpool,
        tc.tile_pool(name="xin", bufs=3) as xpool,
        tc.tile_pool(name="work", bufs=2) as wpool,
        tc.tile_pool(name="acc", bufs=2, space="PSUM") as ppool,
    ):
        # Startup DMAs issue from separate engine queues so they don't
        # serialize on Sync: wm gates the first LDWEIGHTS, xh band 0/1 gate
        # the first matmuls/activations.
        wm = cpool.tile([P, E + 1, P], HDT)
        nc.sync.dma_start(wm[:], wm_d[:])
        pv = cpool.tile([P, 2 * E + 1], F32)
        nc.sync.dma_start(pv[:], pv_d[:])

        r0 = 0
        for band, rb in enumerate(_band_sizes()):
            nseg = rb // SEG
            xbh = xpool.tile([P, rb + 2, WP], HDT, tag="xbh")
            nc.sync.dma_start(xbh[:], xh_d[:, r0 : r0 + rb + 2, :])

            # mean sums per segment (PE accumulation groups) — emitted first
            # so the min-phase unblocks as early as possible
            mean = wpool.tile([P, rb, W], HDT, tag="mean")
            for s in range(nseg):
                i0 = s * SEG
                sp = ppool.tile([P, SEG, W], F32, tag="sp")
                for e, (dy, dx) in enumerate(TAPS):
                    rhs = xbh[
                        :, i0 + 1 + dy : i0 + 1 + dy + SEG, 1 + dx : 1 + dx + W
                    ]
                    nc.tensor.matmul(
                        sp[:],
                        wm[:, e, :],
                        rhs,
                        start=(e == 0),
                        stop=(e == E - 1),
                    )
                # mean = sp/9 - kbar   (Act, PSUM in)
                nc.scalar.activation(
                    mean[:, i0 : i0 + SEG, :],
                    sp[:],
                    Ident,
                    bias=pv[:, 2 * E : 2 * E + 1],
                    scale=1.0 / E,
                )

            # A_e = |m_e * x - k_e| on the tap's shifted window, so all
            # downstream reads are dense/aligned. The last NDVE taps are made
            # on DVE (two 4x tensor_scalar ops via a signed-g intermediate).
            A = []
            for e, (dy, dx) in enumerate(TAPS):
                a = wpool.tile([P, rb, W], HDT, tag=f"A{e}")
                win = xbh[:, 1 + dy : 1 + dy + rb, 1 + dx : 1 + dx + W]
                if e < E - NDVE:
                    nc.scalar.activation(
                        a[:],
                        win,
                        Abs,
                        bias=pv[:, E + e : E + e + 1],
                        scale=pv[:, e : e + 1],
                    )
                else:
                    # signed affine into the A tile, then in-place abs via
                    # sign-bit clear (both 4x tensor_scalar)
                    nc.vector.tensor_scalar(
                        a[:],
                        win,
                        pv[:, e : e + 1],
                        pv[:, E + e : E + e + 1],
                        mult,
                        add,
                    )
                    nc.vector.tensor_scalar(
                        a[:].bitcast(U16), a[:].bitcast(U16), 0x7FFF, None, band_
                    )
                A.append(a)

            # optional DVE pre-sums of A pairs (frees PE passes)
            APs = {}
            for i, (t0, t1) in enumerate(a_pairs):
                pa = wpool.tile([P, rb, W], HDT, tag=f"PA{i}")
                nc.vector.tensor_tensor(pa[:], A[t0][:], A[t1][:], add)
                APs[(t0, t1)] = pa
            norm_fields = [A[0]] + [APs[p] for p in a_pairs] + [
                A[e] for e in range(1, E) if e not in a_paired
            ]

            # norm1 = sum_e A_e on PE; W = (9 - norm1)/9 on Act.
            # Chains run per segment into bank-aligned halves of a 2-bank
            # PSUM tile; W folds once per segment-pair (fewer, bigger ops).
            SW = SEG * W
            w9 = wpool.tile([P, rb, W], HDT, tag="w9")
            for p0 in range(0, nseg, 2):
                nr = min(2, nseg - p0)
                sa2 = ppool.tile([P, 2, 512], F32, tag="sa", bufs=int(os.environ.get("FM_SABUFS", "1")))
                for j in range(nr):
                    i0 = (p0 + j) * SEG
                    for jf, f in enumerate(norm_fields):
                        nc.tensor.matmul(
                            sa2[:, j : j + 1, 0:SW],
                            wm[:, E, :],
                            f[:, i0 : i0 + SEG, :],
                            start=(jf == 0),
                            stop=(jf == len(norm_fields) - 1),
                        )
                nc.scalar.activation(
                    w9[:, p0 * SEG : (p0 + nr) * SEG, :],
                    sa2[:, 0:nr, 0:SW],
                    Copy,
                    bias=1.0,
                    scale=-1.0 / E,
                )

            # min-fields sm_e = min(A_e, mean)  (one 2x TT per tap)
            sm = []
            for e in range(E):
                t = wpool.tile(
                    [P, rb, W],
                    HDT,
                    tag=f"sm{e}",
                    bufs=int(os.environ.get("FM_SMBUFS", "2")),
                )
                nc.vector.tensor_tensor(t[:], A[e][:], mean[:], amin)
                sm.append(t)

            # in-place pair pre-sums: sm[t0] += sm[t1] (mins already done)
            for t0, t1 in s_pairs:
                nc.vector.tensor_tensor(sm[t0][:], sm[t0][:], sm[t1][:], add)
            s_fields = [sm[0]] + [sm[t0] for t0, _ in s_pairs] + [
                sm[e] for e in range(1, E) if e not in s_paired
            ]

            # S sum on PE, then out = W * (W - mean + (2/9) S), finals per
            # segment-pair on DVE
            ob = wpool.tile([P, rb, W], HDT, tag="ob")
            for p0 in range(0, nseg, 2):
                nr = min(2, nseg - p0)
                sv2 = ppool.tile([P, 2, 512], F32, tag="sv", bufs=int(os.environ.get("FM_SVBUFS", "2")))
                for j in range(nr):
                    i0 = (p0 + j) * SEG
                    for jf, f in enumerate(s_fields):
                        nc.tensor.matmul(
                            sv2[:, j : j + 1, 0:SW],
                            wm[:, E, :],
                            f[:, i0 : i0 + SEG, :],
                            start=(jf == 0),
                            stop=(jf == len(s_fields) - 1),
                        )
                i0 = p0 * SEG
                i1 = (p0 + nr) * SEG
                # s1 = (2/9) S - mean   (STT, PSUM in)
                s1 = wpool.tile([P, 2 * SEG, W], HDT, tag="s1")
                nc.vector.scalar_tensor_tensor(
                    s1[:, 0 : i1 - i0, :],
                    sv2[:, 0:nr, 0:SW],
                    2.0 / E,
                    mean[:, i0:i1, :],
                    mult,
                    sub,
                )
                # s2 = W + s1 ; out = W * s2   (two 2x TTs)
                s2 = wpool.tile([P, 2 * SEG, W], HDT, tag="s2")
                nc.vector.tensor_tensor(
                    s2[:, 0 : i1 - i0, :], w9[:, i0:i1, :], s1[:, 0 : i1 - i0, :], add
                )
                nc.vector.tensor_tensor(
                    ob[:, i0:i1, :], w9[:, i0:i1, :], s2[:, 0 : i1 - i0, :], mult
                )
            nc.sync.dma_start(y_d[:, r0 : r0 + rb, :], ob[:])
            r0 += rb


def _host_pack(inp, kern, mask):
    """Build per-core input maps."""
    inp = np.ascontiguousarray(inp, dtype=np.float32)
    kern = np.asarray(kern, dtype=np.float32).reshape(E, C)
    mask = np.asarray(mask, dtype=np.float32).reshape(E, C)

    m = np.abs(mask) / (np.abs(mask).max() + np.float32(1e-6))  # [E,C]
    kbar = kern.mean(axis=0)  # [C]

    cidx = np.arange(P) % C
    pv = np.empty((P, 2 * E + 1), np.float32)
    for e in range(E):
        pv[:, e] = m[e][cidx]
        pv[:, E + e] = -kern[e][cidx]
    pv[:, 2 * E] = -kbar[cidx]

    wm = np.zeros((P, E + 1, P), np.float16)
    rng = np.arange(P)
    for e in range(E):
        wm[rng, e, rng] = m[e][cidx]
    wm[rng, E, rng] = 1.0

    in_maps = []
    for b in range(NCORES):
        padded = np.pad(inp[b], ((1, 1), (1, 1), (0, 0)))  # [226,226,32]
        # quarters: q needs padded rows [56q, 56q+58)
        qs = np.stack(
            [padded[RQ * q : RQ * q + RA] for q in range(Q)], axis=0
        )  # [4,58,226,32]
        x_dev = np.ascontiguousarray(
            qs.transpose(0, 3, 1, 2).reshape(P, RA, WP)
        )
        in_maps.append(
            {
                "xh": x_dev.astype(np.float16),
                "pv": pv,
                "wm": wm,
            }
        )
    return in_maps


def _host_unpack(results):
    out = np.empty((B, H, W, C), np.float32)
    for b in range(NCORES):
        y = results[b]["y"].astype(np.float32).reshape(Q, C, RQ, W)
        out[b] = y.transpose(0, 2, 3, 1).reshape(H, W, C)
    return out


LAST_PROFILE = {}


def _install_ntff_shim():
    """antenv.axon_hooks is missing in this image; synthesize it so
    run_bass_kernel_spmd(trace=True) can capture NTFF profiles."""
    import contextlib
    import ctypes
    import types

    if "antenv.axon_hooks" in sys.modules:
        return
    so_path = "/opt/axon/libaxon_pjrt.so"
    try:
        lib = ctypes.CDLL(so_path)
    except OSError:
        return
    if not hasattr(lib, "axon_start_nrt_profile"):
        return
    lib.axon_start_nrt_profile.argtypes = [
        ctypes.POINTER(ctypes.c_int64),
        ctypes.c_size_t,
    ]
    lib.axon_start_nrt_profile.restype = ctypes.c_int64
    lib.axon_stop_nrt_profile.argtypes = [ctypes.c_char_p]
    lib.axon_stop_nrt_profile.restype = ctypes.c_int64

    @contextlib.contextmanager
    def _hook(output_dir, device_ids):
        import jax

        jax.devices()
        if device_ids:
            ids = (ctypes.c_int64 * len(device_ids))(*device_ids)
            rc = lib.axon_start_nrt_profile(ids, len(device_ids))
        else:
            rc = lib.axon_start_nrt_profile(None, 0)
        if rc != 0:
            raise RuntimeError(f"axon_start_nrt_profile rc={rc}")
        try:
            yield
        finally:
            n = lib.axon_stop_nrt_profile(str(output_dir).encode())
            if n < 0:
                raise RuntimeError(f"axon_stop_nrt_profile rc={n}")
            print(f"ntff profile: {n} file(s) written to {output_dir}")

    mod = types.ModuleType("antenv.axon_hooks")
    mod._hook = _hook
    mod.get_axon_ntff_profile_hook = lambda: mod._hook
    mod.set_axon_ntff_profile_hook = lambda h: setattr(mod, "_hook", h)
    sys.modules["antenv.axon_hooks"] = mod


def kernel(inp, kernel, mask):
    if "nc" not in _CACHE:
        _CACHE["nc"] = _build_program()
    nc = _CACHE["nc"]

    in_maps = _host_pack(inp, kernel, mask)
    trace = bool(int(os.environ.get("FM_TRACE", "0")))
    if trace:
        _install_ntff_shim()
    res = run_bass_kernel_spmd(
        nc, in_maps, core_ids=list(range(NCORES)), trace=trace
    )
    LAST_PROFILE["exec_time_ns"] = res.exec_time_ns
    LAST_PROFILE["mean_exec_time_ns"] = res.mean_exec_time_ns
    return _host_unpack(res.results)


# revision 23
# speedup vs baseline: 1.0289x; 1.0045x over previous
"""Trainium2 Bass kernel for nn_FactorMask (9-tap masked-stencil op).

Contract: kernel(**inputs) takes FULL inputs (inp [8,224,224,32] f32,
kernel [9,1,1,1,32], mask [9,1,1,1,32]) and returns the FULL output
[8,224,224,32] f32. Internally: batch-parallel across 8 NeuronCores
(1 image per core), host-side repack to a channels-on-partition layout,
Bass/Tile kernel per core, host-side gather.

Layout per core: xh [128, 58, 226] fp16 where partition p = 32*q + c
(q = H-quarter 0..3, c = channel), rows = 58 padded rows of that quarter
(56 output rows + 1 halo row each side), cols = 226 zero-padded W.
All 9 stencil taps are then pure free-dim AP offsets.

Math (per pixel/channel): A_e = |m_e x_e - k_e|, mu = (sum m_e x_e)/9 - kbar,
norm1 = sum A_e, var = sum |A_e - mu|. Using |A-mu| = A + mu - 2*min(A,mu):
    var = norm1 + 9 mu - 2 S,   S = sum_e min(A_e, mu)
    out = (1 - var/9)(1 - norm1/9) = W * (W - mu + (2/9) S),  W = (9-norm1)/9
so the var path needs only one 2x tensor_tensor(min) per tap instead of
subtract+abs.

Work split (v3):
- Act: A_e for taps 0..5 (fused abs-affine), mean affine, W fold
- DVE: A_e for taps 6..8 (two 4x tensor_scalar ops), min-fields (2x),
  optional pair pre-sums, final combine
- PE: mean conv (9 diag matmuls) + norm1 + S sums per segment
"""

import os
import sys

for _p in ("/opt/trn_rl_repo", "/opt/pypackages"):
    if _p not in sys.path:
        sys.path.insert(0, _p)

import numpy as np

import concourse.bacc as bacc
import concourse.mybir as mybir
import concourse.tile as tile
from concourse.bass_utils import run_bass_kernel_spmd

# ---- problem constants (hardcoded per the task spec) ----
B, H, W, C = 8, 224, 224, 32
E = 9  # taps
NCORES = 8
Q = 4  # H-quarters per image -> 4*32 = 128 partitions
RQ = H // Q  # 56 output rows per quarter
RA = RQ + 2  # 58 rows incl halo
WP = W + 2  # 226 padded cols
P = 128

# tap order must match reference: element 0 = center, then (y,x) raster
# order skipping center, with shift (dy,dx) = (y-1, x-1)
TAPS = [(0, 0)] + [
    (dy, dx) for dy in (-1, 0, 1) for dx in (-1, 0, 1) if not (dy == 0 and dx == 0)
]

# ---- tunables ----
SEG = 2  # rows per matmul segment (2*224 = 448 <= 512)
NDVE = int(os.environ.get("FM_NDVE", "2"))  # taps whose A is made on DVE
NPAIR_A = int(os.environ.get("FM_NPAIR_A", "1"))  # DVE pre-sum pairs (norm1)
NPAIR_S = int(os.environ.get("FM_NPAIR_S", "2"))  # DVE pre-sum pairs (S)
F32 = mybir.dt.float32
HDT = mybir.dt.float16
U16 = mybir.dt.uint16

_CACHE = {}


def _build_program():
    nc = bacc.Bacc(
        "TRN2", target_bir_lowering=False, debug=False, num_devices=NCORES
    )
    xh_d = nc.dram_tensor("xh", [P, RA, WP], HDT, kind="ExternalInput").ap()
    # pv columns: [0:9]=m_e scale, [9:18]=-k_e bias, [18]=-kbar
    pv_d = nc.dram_tensor("pv", [P, 2 * E + 1], F32, kind="ExternalInput").ap()
    # wm[:, e, :] = diag(m_e) for e<9 ; wm[:, 9, :] = identity  (fp16)
    wm_d = nc.dram_tensor("wm", [P, E + 1, P], HDT, kind="ExternalInput").ap()
    y_d = nc.dram_tensor("y", [P, RQ, W], HDT, kind="ExternalOutput").ap()

    with tile.TileContext(nc) as tc:
        _emit(tc, nc, xh_d, pv_d, wm_d, y_d)
    nc.compile()
    return nc


def _band_sizes():
    sched = os.environ.get("FM_BANDS", "2,4,8,8,8,8,8,8,2")
    sizes = [int(s) for s in sched.split(",")]
    assert sum(sizes) == RQ and all(s % SEG == 0 for s in sizes)
    return sizes


def _emit(tc, nc, xh_d, pv_d, wm_d, y_d):
    Abs = mybir.ActivationFunctionType.Abs
    Ident = mybir.ActivationFunctionType.Identity
    Copy = mybir.ActivationFunctionType.Copy
    sub = mybir.AluOpType.subtract
    mult = mybir.AluOpType.mult
    add = mybir.AluOpType.add
    amin = mybir.AluOpType.min
    band_ = mybir.AluOpType.bitwise_and

    a_pairs = [(1 + 2 * i, 2 + 2 * i) for i in range(NPAIR_A)]
    s_pairs = [(1 + 2 * i, 2 + 2 * i) for i in range(NPAIR_S)]
    a_paired = {t for p in a_pairs for t in p}
    s_paired = {t for p in s_pairs for t in p}

    with (
        tc.tile_pool(name="const", bufs=1) as cpool,
        tc.tile_pool(name="xin", bufs=3) as xpool,
        tc.tile_pool(name="work", bufs=2) as wpool,
        tc.tile_pool(name="acc", bufs=2, space="PSUM") as ppool,
    ):
        # Startup DMAs issue from separate engine queues so they don't
        # serialize on Sync: wm gates the first LDWEIGHTS, xh band 0/1 gate
        # the first matmuls/activations.
        wm = cpool.tile([P, E + 1, P], HDT)
        nc.sync.dma_start(wm[:], wm_d[:])
        pv = cpool.tile([P, 2 * E + 1], F32)
        nc.sync.dma_start(pv[:], pv_d[:])

        r0 = 0
        for band, rb in enumerate(_band_sizes()):
            nseg = rb // SEG
            xbh = xpool.tile([P, rb + 2, WP], HDT, tag="xbh")
            nc.sync.dma_start(xbh[:], xh_d[:, r0 : r0 + rb + 2, :])

            # mean sums per segment (PE accumulation groups) — emitted first
            # so the min-phase unblocks as early as possible
            mean = wpool.tile([P, rb, W], HDT, tag="mean")
            for s in range(nseg):
                i0 = s * SEG
                sp = ppool.tile([P, SEG, W], F32, tag="sp")
                for e, (dy, dx) in enumerate(TAPS):
                    rhs = xbh[
                        :, i0 + 1 + dy : i0 + 1 + dy + SEG, 1 + dx : 1 + dx + W
                    ]
                    nc.tensor.matmul(
                        sp[:],
                        wm[:, e, :],
                        rhs,
                        start=(e == 0),
                        stop=(e == E - 1),
                    )
                # mean = sp/9 - kbar   (Act, PSUM in)
                nc.scalar.activation(
                    mean[:, i0 : i0 + SEG, :],
                    sp[:],
                    Ident,
                    bias=pv[:, 2 * E : 2 * E + 1],
                    scale=1.0 / E,
                )

            # A_e = |m_e * x - k_e| on the tap's shifted window, so all
            # downstream reads are dense/aligned. The last NDVE taps are made
            # on DVE (two 4x tensor_scalar ops via a signed-g intermediate).
            A = []
            for e, (dy, dx) in enumerate(TAPS):
                a = wpool.tile([P, rb, W], HDT, tag=f"A{e}")
                win = xbh[:, 1 + dy : 1 + dy + rb, 1 + dx : 1 + dx + W]
                if e < E - NDVE:
                    nc.scalar.activation(
                        a[:],
                        win,
                        Abs,
                        bias=pv[:, E + e : E + e + 1],
                        scale=pv[:, e : e + 1],
                    )
                else:
                    # signed affine into the A tile, then in-place abs via
                    # sign-bit clear (both 4x tensor_scalar)
                    nc.vector.tensor_scalar(
                        a[:],
                        win,
                        pv[:, e : e + 1],
                        pv[:, E + e : E + e + 1],
                        mult,
                        add,
                    )
                    nc.vector.tensor_scalar(
                        a[:].bitcast(U16), a[:].bitcast(U16), 0x7FFF, None, band_
                    )
                A.append(a)

            # optional DVE pre-sums of A pairs (frees PE passes)
            APs = {}
            for i, (t0, t1) in enumerate(a_pairs):
                pa = wpool.tile([P, rb, W], HDT, tag=f"PA{i}")
                nc.vector.tensor_tensor(pa[:], A[t0][:], A[t1][:], add)
                APs[(t0, t1)] = pa
            norm_fields = [A[0]] + [APs[p] for p in a_pairs] + [
                A[e] for e in range(1, E) if e not in a_paired
            ]

            # norm1 = sum_e A_e on PE; W = (9 - norm1)/9 on Act.
            # Chains run per segment into bank-aligned halves of a 2-bank
            # PSUM tile; W folds once per segment-pair (fewer, bigger ops).
            SW = SEG * W
            w9 = wpool.tile([P, rb, W], HDT, tag="w9")
            for p0 in range(0, nseg, 2):
                nr = min(2, nseg - p0)
                sa2 = ppool.tile([P, 2, 512], F32, tag="sa", bufs=int(os.environ.get("FM_SABUFS", "1")))
                for j in range(nr):
                    i0 = (p0 + j) * SEG
                    for jf, f in enumerate(norm_fields):
                        nc.tensor.matmul(
                            sa2[:, j : j + 1, 0:SW],
                            wm[:, E, :],
                            f[:, i0 : i0 + SEG, :],
                            start=(jf == 0),
                            stop=(jf == len(norm_fields) - 1),
                        )
                nc.scalar.activation(
                    w9[:, p0 * SEG : (p0 + nr) * SEG, :],
                    sa2[:, 0:nr, 0:SW],
                    Copy,
                    bias=1.0,
                    scale=-1.0 / E,
                )

            # min-fields sm_e = min(A_e, mean)  (one 2x TT per tap)
            sm = []
            for e in range(E):
                t = wpool.tile(
                    [P, rb, W],
                    HDT,
                    tag=f"sm{e}",
                    bufs=int(os.environ.get("FM_SMBUFS", "2")),
                )
                nc.vector.tensor_tensor(t[:], A[e][:], mean[:], amin)
                sm.append(t)

            # in-place pair pre-sums: sm[t0] += sm[t1] (mins already done).
            # FM_GPAIR=1 runs them on the otherwise-idle gpsimd engine.
            pair_eng = nc.gpsimd if int(os.environ.get("FM_GPAIR", "0")) else nc.vector
            for t0, t1 in s_pairs:
                pair_eng.tensor_tensor(sm[t0][:], sm[t0][:], sm[t1][:], add)
            s_fields = [sm[0]] + [sm[t0] for t0, _ in s_pairs] + [
                sm[e] for e in range(1, E) if e not in s_paired
            ]

            # S sum on PE, then out = W * (W - mean + (2/9) S), finals per
            # segment-pair on DVE
            ob = wpool.tile([P, rb, W], HDT, tag="ob")
            for p0 in range(0, nseg, 2):
                nr = min(2, nseg - p0)
                sv2 = ppool.tile([P, 2, 512], F32, tag="sv", bufs=int(os.environ.get("FM_SVBUFS", "2")))
                for j in range(nr):
                    i0 = (p0 + j) * SEG
                    for jf, f in enumerate(s_fields):
                        nc.tensor.matmul(
                            sv2[:, j : j + 1, 0:SW],
                            wm[:, E, :],
                            f[:, i0 : i0 + SEG, :],
                            start=(jf == 0),
                            stop=(jf == len(s_fields) - 1),
                        )
                i0 = p0 * SEG
                i1 = (p0 + nr) * SEG
                # s1 = (2/9) S - mean   (STT, PSUM in)
                s1 = wpool.tile([P, 2 * SEG, W], HDT, tag="s1")
                nc.vector.scalar_tensor_tensor(
                    s1[:, 0 : i1 - i0, :],
                    sv2[:, 0:nr, 0:SW],
                    2.0 / E,
                    mean[:, i0:i1, :],
                    mult,
                    sub,
                )
                # s2 = W + s1 ; out = W * s2   (two 2x TTs)
                s2 = wpool.tile([P, 2 * SEG, W], HDT, tag="s2")
                nc.vector.tensor_tensor(
                    s2[:, 0 : i1 - i0, :], w9[:, i0:i1, :], s1[:, 0 : i1 - i0, :], add
                )
                nc.vector.tensor_tensor(
                    ob[:, i0:i1, :], w9[:, i0:i1, :], s2[:, 0 : i1 - i0, :], mult
                )
            nc.sync.dma_start(y_d[:, r0 : r0 + rb, :], ob[:])
            r0 += rb


def _host_pack(inp, kern, mask):
    """Build per-core input maps."""
    inp = np.ascontiguousarray(inp, dtype=np.float32)
    kern = np.asarray(kern, dtype=np.float32).reshape(E, C)
    mask = np.asarray(mask, dtype=np.float32).reshape(E, C)

    m = np.abs(mask) / (np.abs(mask).max() + np.float32(1e-6))  # [E,C]
    kbar = kern.mean(axis=0)  # [C]

    cidx = np.arange(P) % C
    pv = np.empty((P, 2 * E + 1), np.float32)
    for e in range(E):
        pv[:, e] = m[e][cidx]
        pv[:, E + e] = -kern[e][cidx]
    pv[:, 2 * E] = -kbar[cidx]

    wm = np.zeros((P, E + 1, P), np.float16)
    rng = np.arange(P)
    for e in range(E):
        wm[rng, e, rng] = m[e][cidx]
    wm[rng, E, rng] = 1.0

    in_maps = []
    for b in range(NCORES):
        padded = np.pad(inp[b], ((1, 1), (1, 1), (0, 0)))  # [226,226,32]
        # quarters: q needs padded rows [56q, 56q+58)
        qs = np.stack(
            [padded[RQ * q : RQ * q + RA] for q in range(Q)], axis=0
        )  # [4,58,226,32]
        x_dev = np.ascontiguousarray(
            qs.transpose(0, 3, 1, 2).reshape(P, RA, WP)
        )
        in_maps.append(
            {
                "xh": x_dev.astype(np.float16),
                "pv": pv,
                "wm": wm,
            }
        )
    return in_maps


def _host_unpack(results):
    out = np.empty((B, H, W, C), np.float32)
    for b in range(NCORES):
        y = results[b]["y"].astype(np.float32).reshape(Q, C, RQ, W)
        out[b] = y.transpose(0, 2, 3, 1).reshape(H, W, C)
    return out


LAST_PROFILE = {}


def _install_ntff_shim():
    """antenv.axon_hooks is missing in this image; synthesize it so
    run_bass_kernel_spmd(trace=True) can capture NTFF profiles."""
    import contextlib
    import ctypes
    import types

    if "antenv.axon_hooks" in sys.modules:
        return
    so_path = "/opt/axon/libaxon_pjrt.so"
    try:
        lib = ctypes.CDLL(so_path)
    except OSError:
        return
    if not hasattr(lib, "axon_start_nrt_profile"):
        return
    lib.axon_start_nrt_profile.argtypes = [
        ctypes.POINTER(ctypes.c_int64),
        ctypes.c_size_t,
    ]
    lib.axon_start_nrt_profile.restype = ctypes.c_int64
    lib.axon_stop_nrt_profile.argtypes = [ctypes.c_char_p]
    lib.axon_stop_nrt_profile.restype = ctypes.c_int64

    @contextlib.contextmanager
    def _hook(output_dir, device_ids):
        import jax

        jax.devices()
        if device_ids:
            ids = (ctypes.c_int64 * len(device_ids))(*device_ids)
            rc = lib.axon_start_nrt_profile(ids, len(device_ids))
        else:
            rc = lib.axon_start_nrt_profile(None, 0)
        if rc != 0:
            raise RuntimeError(f"axon_start_nrt_profile rc={rc}")
        try:
            yield
        finally:
            n = lib.axon_stop_nrt_profile(str(output_dir).encode())
            if n < 0:
                raise RuntimeError(f"axon_stop_nrt_profile rc={n}")
            print(f"ntff profile: {n} file(s) written to {output_dir}")

    mod = types.ModuleType("antenv.axon_hooks")
    mod._hook = _hook
    mod.get_axon_ntff_profile_hook = lambda: mod._hook
    mod.set_axon_ntff_profile_hook = lambda h: setattr(mod, "_hook", h)
    sys.modules["antenv.axon_hooks"] = mod


def kernel(inp, kernel, mask):
    if "nc" not in _CACHE:
        _CACHE["nc"] = _build_program()
    nc = _CACHE["nc"]

    in_maps = _host_pack(inp, kernel, mask)
    trace = bool(int(os.environ.get("FM_TRACE", "0")))
    if trace:
        _install_ntff_shim()
    res = run_bass_kernel_spmd(
        nc, in_maps, core_ids=list(range(NCORES)), trace=trace
    )
    LAST_PROFILE["exec_time_ns"] = res.exec_time_ns
    LAST_PROFILE["mean_exec_time_ns"] = res.mean_exec_time_ns
    return _host_unpack(res.results)
